# revision 7
# baseline (speedup 1.0000x reference)
"""Bass/Trainium2 kernel for nn_Attention_6682969112611.

Math (faithful to the buggy torch module):
    k_t   = k.reshape(b, l, c)                  # row-major reshape, NOT a transpose
    score = (q @ k_t) / sqrt(l)                 # (b, c, c)
    score = softmax(score, axis=0)              # softmax over the BATCH axis
    out   = score @ v                           # (b, c, l)

B=16, C=2048, L=64. Sharding: the c (query-row) axis of q/score/out is split
across 8 cores (256 rows each); k and v are replicated. The batch-axis softmax
needs, for every (c, c') pair, all 16 batch values - which all live on the same
core under c-sharding, so there are no collectives. Different c' columns are
independent, so we stream over c' in chunks of 128 and accumulate the second
matmul in PSUM.

Engine budget per c' chunk (the kernel is elementwise-bound, ACT is pacer):
  ACT   : 4x exp instrs (N=1024, PSUM->SBUF bf16)        ~4.4 us
  DVE   : tree L1 (A,Bt) + recip + cast + en=e*r         ~4.3 us
  GPSIMD: tree mid levels (C=A+Bt, D1, d)                ~3.9 us
  PE    : mm1 row-tiled 2x (K=64 pairs in T0/T8) and
          mm2 col-tiled 2x (v-stationary, M=64 pairs)    ~3.5 us cold
  DMA   : k chunk 0.5MB + v chunk 0.5MB                  ~2.9 us

mm1 row tiling: batches are laid out host-side as p = 64*h + l with
h = (b%4)//2, so each 4-batch PSUM group issues pairs (T0-half, T8-half)
that land in DIFFERENT psum banks (slot s=2h+r, bank=s//2) - concurrent
row-tile drains into one bank are fatal, this layout avoids them.

mm2 col tiling: v[c',l] is the stationary operand (64 weight cols), en
streams (N=256); batch pairs (2t, 2t+1) write partitions 0-63 / 64-127 of
pair-tile t. PSUM acc banks are pre-cleared by 4 dummy matmuls so every
real mm2 runs start=False (overwrite-where-clear handles chunk 0).
"""

import os

import numpy as np
import ml_dtypes

B, C, L = 16, 2048, 64
NCORES = 8
CB = C // NCORES  # 256 query rows per core
NJ = 16           # c' chunks of 128
P = 128

# debug bisect knobs (comma-separated): gp0 (tree all on DVE), oldmm2
# (en-stationary non-col-tiled mm2), flatmm1 (no row-tile pairing: issue
# order s=0,1,2,3 instead of 0,2,1,3)
_VARIANT = set(filter(None, os.environ.get("KERNEL_VARIANT", "").split(",")))

_NC_CACHE: dict = {}


def _build_nc():
    import concourse.mybir as mybir
    import concourse.tile as tile
    from concourse import bacc

    f32 = mybir.dt.float32
    bf16 = mybir.dt.bfloat16
    Exp = mybir.ActivationFunctionType.Exp
    ADD = mybir.AluOpType.add
    MUL = mybir.AluOpType.mult

    nc = bacc.Bacc(None, target_bir_lowering=False, debug=False)

    use_gp = "gp0" not in _VARIANT
    oldmm2 = "oldmm2" in _VARIANT

    # qt[p, m, cq]: p = 64*h + l (h=(b%4)//2), m = 2*(b//4) + b%2
    qt = nc.declare_dram_parameter("qt", [P, 8, CB], bf16, isOutput=False)
    # kt[j, p, m, c']: same (p, m) mapping as qt
    kt = nc.declare_dram_parameter("kt", [NJ, P, 8, 128], bf16, isOutput=False)
    # vt[j, c', b, l]  (16, 128, 16, 64)
    vt = nc.declare_dram_parameter("vt", [NJ, P, B, L], bf16, isOutput=False)
    if oldmm2:
        # outd[t, cq_lo, bq, h, l]: b = 4t + bq, cq = 128h + cq_lo
        outd = nc.declare_dram_parameter("outd", [4, P, 4, 2, L], f32, isOutput=True)
    else:
        # outd[p, t, cq]: b = 2t + p//64, l = p%64
        outd = nc.declare_dram_parameter("outd", [P, 8, CB], f32, isOutput=True)

    with tile.TileContext(nc) as tc:
        with (
            tc.tile_pool(name="qp", bufs=1) as qp,
            tc.tile_pool(name="kp", bufs=4) as kp,
            tc.tile_pool(name="vp", bufs=4) as vp,
            tc.tile_pool(name="ep", bufs=4) as ep,
            tc.tile_pool(name="enp", bufs=3) as enp,
            tc.tile_pool(name="tp", bufs=3) as tp,
            tc.tile_pool(name="dp", bufs=3) as dp,
            tc.tile_pool(name="osp", bufs=4) as osp,
            tc.tile_pool(name="mm1p", bufs=2, space="PSUM") as mm1p,
            tc.tile_pool(name="accp", bufs=1, space="PSUM") as accp,
        ):
            qt_s = qp.tile([P, 8, CB], bf16)
            for g in range(4):
                nc.sync.dma_start(out=qt_s[:, 2 * g : 2 * g + 2], in_=qt[:, 2 * g : 2 * g + 2])

            if oldmm2:
                accs_old = [accp.tile([P, 4, 2, L], f32, name=f"acc{t}") for t in range(4)]
                accs = None
            else:
                accs = accp.tile([P, 8, CB], f32, name="acc")

            # HAM warmup: dense zero matmuls so the PE clock is at 8/8 by the
            # time real work lands; runs under the initial k/v DMAs.
            nwarm = int(os.environ.get("KERNEL_NWARM", "14"))
            wseed = qp.tile([P, 512], bf16, name="wseed")
            nc.vector.memset(wseed[:], 0)
            if nwarm:
                wps = mm1p.tile([P, 4, CB], f32, name="ps")
                for i in range(nwarm):
                    s = (0, 2, 1, 3)[i % 4]
                    h = s // 2
                    nc.tensor.matmul(
                        wps[:, s],
                        lhsT=wseed[64 * h : 64 * h + 64, :128],
                        rhs=wseed[64 * h : 64 * h + 64, :CB],
                        start=True,
                        stop=True,
                    )

            if not oldmm2:
                # pre-clear the 4 acc banks: one dummy start=True matmul per
                # bank clears its has_written bits, so all real mm2 matmuls
                # use start=False (overwrite-where-clear == accumulate-from-0)
                for u in range(4):
                    nc.tensor.matmul(
                        accs[:, 2 * u, 0:1],
                        lhsT=wseed[0:64, 0:128],
                        rhs=wseed[0:64, 0:1],
                        start=True,
                        stop=False,
                        skip_group_check=True,
                    )

            def emit_mm2_pair(j, en_j, v_j, t):
                # col-tiled pair: batch 2t -> out partitions 0-63 (tile (0,0)),
                # batch 2t+1 -> 64-127 (tile (0,64)); both into bank 4 + t//2.
                last = j == NJ - 1
                for p_ in range(2):
                    b = 2 * t + p_
                    nc.tensor.matmul(
                        accs[64 * p_ : 64 * p_ + 64, t, :],
                        lhsT=v_j[:, b],
                        rhs=en_j[:, b],
                        start=False,
                        stop=last and p_ == 1 and t % 2 == 1,
                        skip_group_check=True,
                    )

            def emit_mm2_b_old(j, en_j, v_j, b):
                acc = accs_old[b // 4]
                for h in range(2):
                    first_in_bank = j == 0 and b % 4 == 0 and h == 0
                    last_in_bank = j == NJ - 1 and b % 4 == 3 and h == 1
                    nc.tensor.matmul(
                        acc[:, b % 4, h],
                        lhsT=en_j[:, b, h * 128 : (h + 1) * 128],
                        rhs=v_j[:, b],
                        start=first_in_bank,
                        stop=last_in_bank,
                        skip_group_check=not (first_in_bank or last_in_bank),
                    )

            def emit_mm2(j, en_j, v_j):
                if oldmm2:
                    for b in range(B):
                        emit_mm2_b_old(j, en_j, v_j, b)
                else:
                    for t in range(8):
                        emit_mm2_pair(j, en_j, v_j, t)

            sorder = (0, 1, 2, 3) if "flatmm1" in _VARIANT else (0, 2, 1, 3)

            pending = None  # software pipeline: mm2 for chunk j-1 is emitted
            # during chunk j so the PE never waits on the softmax chain
            for j in range(NJ):
                k_j = kp.tile([P, 8, 128], bf16, name="k_j")
                nc.sync.dma_start(out=k_j[:], in_=kt[j])
                v_j = vp.tile([P, B, L], bf16, name="v_j")
                nc.sync.dma_start(out=v_j[:], in_=vt[j])

                e_j = ep.tile([P, B, CB], bf16, name="e_j")
                A_t = tp.tile([P, 4, CB], bf16, name="A_t")
                B_t = tp.tile([P, 4, CB], bf16, name="B_t")
                for g in range(4):
                    ps = mm1p.tile([P, 4, CB], f32, name="ps")
                    # slot s = 2h + r -> batch 4g + s; pairs (s=0,2) and
                    # (s=1,3) run concurrently on row tiles T0/T8 and write
                    # different psum banks (s//2).
                    for s in sorder:
                        h, r = s // 2, s % 2
                        nc.tensor.matmul(
                            ps[:, s],
                            lhsT=k_j[64 * h : 64 * h + 64, 2 * g + r],
                            rhs=qt_s[64 * h : 64 * h + 64, 2 * g + r],
                            start=True,
                            stop=True,
                        )
                    nc.scalar.activation(
                        e_j[:, 4 * g : 4 * g + 4], ps[:], Exp, scale=0.125
                    )
                    if g == 1:
                        nc.vector.tensor_tensor(
                            A_t[:], e_j[:, 0:4], e_j[:, 4:8], ADD
                        )
                if pending is not None:
                    emit_mm2(*pending)

                nc.vector.tensor_tensor(B_t[:], e_j[:, 8:12], e_j[:, 12:16], ADD)

                # mid tree levels on GPSIMD (idle engine) except at the tail,
                # where chain latency matters and DVE is free
                tteng = nc.gpsimd if (use_gp and j < NJ - 1) else nc.vector
                C_t = tp.tile([P, 4, CB], bf16, name="C_t")
                tteng.tensor_tensor(C_t[:], A_t[:], B_t[:], ADD)
                D_t = tp.tile([P, 2, CB], bf16, name="D_t")
                tteng.tensor_tensor(D_t[:], C_t[:, 0:2], C_t[:, 2:4], ADD)
                d_f = dp.tile([P, CB], f32, name="d_f")
                tteng.tensor_tensor(d_f[:], D_t[:, 0], D_t[:, 1], ADD)

                r_f = dp.tile([P, CB], f32, name="r_f")
                nc.vector.reciprocal_approx_fast(r_f[:], d_f[:])
                r_b = dp.tile([P, CB], bf16, name="r_b")
                nc.vector.tensor_copy(out=r_b[:], in_=r_f[:])

                en_j = enp.tile([P, B, CB], bf16, name="en_j")
                if j == NJ - 1:
                    # tail: normalize in two halves, each followed by its mm2
                    # block; each acc bank's psum->sbuf copy + store DMA is
                    # emitted as soon as that bank's accumulation completes
                    for gh in range(2):
                        nc.vector.tensor_tensor(
                            en_j[:, 8 * gh : 8 * gh + 8],
                            e_j[:, 8 * gh : 8 * gh + 8],
                            r_b[:, None, :].to_broadcast((P, 8, CB)),
                            MUL,
                        )
                        if oldmm2:
                            for b in range(8 * gh, 8 * gh + 8):
                                emit_mm2_b_old(j, en_j, v_j, b)
                                if b % 4 == 3:
                                    t = b // 4
                                    o_s = osp.tile([P, 4, 2, L], f32, name="o_s")
                                    if t % 2 == 0:
                                        nc.vector.tensor_copy(
                                            out=o_s[:], in_=accs_old[t][:]
                                        )
                                    else:
                                        nc.scalar.copy(o_s[:], accs_old[t][:])
                                    nc.sync.dma_start(out=outd[t], in_=o_s[:])
                        else:
                            for t in range(4 * gh, 4 * gh + 4):
                                emit_mm2_pair(j, en_j, v_j, t)
                                if t % 2 == 1:
                                    u = t // 2
                                    o_s = osp.tile([P, 2, CB], f32, name="o_s")
                                    if u % 2 == 0:
                                        nc.vector.tensor_copy(
                                            out=o_s[:],
                                            in_=accs[:, 2 * u : 2 * u + 2],
                                        )
                                    else:
                                        nc.scalar.copy(
                                            o_s[:], accs[:, 2 * u : 2 * u + 2]
                                        )
                                    nc.sync.dma_start(
                                        out=outd[:, 2 * u : 2 * u + 2], in_=o_s[:]
                                    )
                else:
                    nc.vector.tensor_tensor(
                        en_j[:],
                        e_j[:],
                        r_b[:, None, :].to_broadcast((P, B, CB)),
                        MUL,
                    )
                    pending = (j, en_j, v_j)

    nc.compile()
    return nc


def get_nc():
    if "nc" not in _NC_CACHE:
        _NC_CACHE["nc"] = _build_nc()
    return _NC_CACHE["nc"]


def make_in_maps(q, k, v):
    q = np.asarray(q, dtype=np.float32)
    k = np.asarray(k, dtype=np.float32)
    v = np.asarray(v, dtype=np.float32)

    qb = q.astype(ml_dtypes.bfloat16)
    kb = k.astype(ml_dtypes.bfloat16)
    # q (b, cq, l) -> qt[64h+l, 2g+r, cq] with b = 4g + 2h + r
    qt_all = np.ascontiguousarray(
        qb.reshape(4, 2, 2, C, L).transpose(1, 4, 0, 2, 3)
    ).reshape(P, 8, C)
    # k -> k_t[b, l, cfull] (row-major reshape) -> kt[j, 64h+l, 2g+r, c']
    ktt = np.ascontiguousarray(
        kb.reshape(4, 2, 2, L, NJ, 128).transpose(4, 1, 3, 0, 2, 5)
    ).reshape(NJ, P, 8, 128)
    # v -> bf16, (c', b, l) -> (j, c'128, b, l)
    vbt = np.ascontiguousarray(
        v.astype(ml_dtypes.bfloat16).transpose(1, 0, 2)
    ).reshape(NJ, P, B, L)

    in_maps = []
    for g in range(NCORES):
        in_maps.append(
            {
                "qt": np.ascontiguousarray(qt_all[:, :, g * CB : (g + 1) * CB]),
                "kt": ktt,
                "vt": vbt,
            }
        )
    return in_maps


def assemble_out(results):
    out = np.empty((B, C, L), dtype=np.float32)
    oldmm2 = "oldmm2" in _VARIANT
    for g in range(NCORES):
        od = np.asarray(results[g]["outd"])
        if oldmm2:
            oc = od.transpose(0, 2, 3, 1, 4).reshape(B, CB, L)
        else:
            # od[p, t, cq]: b = 2t + p//64, l = p%64
            oc = (
                od.reshape(2, L, 8, CB).transpose(2, 0, 3, 1).reshape(B, CB, L)
            )
        out[:, g * CB : (g + 1) * CB, :] = oc
    return out


def run(q, k, v, trace=False, trace_kwargs=None):
    """Run on 8 NeuronCores; returns (out, BassKernelResults)."""
    from concourse.bass_utils import run_bass_kernel_spmd

    nc = get_nc()
    in_maps = make_in_maps(q, k, v)
    kwargs = {}
    if trace:
        kwargs["trace"] = True
        if trace_kwargs:
            kwargs["trace_kwargs"] = trace_kwargs
    res = run_bass_kernel_spmd(nc, in_maps, core_ids=list(range(NCORES)), **kwargs)
    return assemble_out(res.results), res


def kernel(q, k, v):
    out, _ = run(q, k, v, trace=False)
    return out


# revision 10
# speedup vs baseline: 1.6099x; 1.6099x over previous
"""Bass/Trainium2 kernel for nn_Attention_6682969112611.

Math (faithful to the buggy torch module):
    k_t   = k.reshape(b, l, c)                  # row-major reshape, NOT a transpose
    score = (q @ k_t) / sqrt(l)                 # (b, c, c)
    score = softmax(score, axis=0)              # softmax over the BATCH axis
    out   = score @ v                           # (b, c, l)

B=16, C=2048, L=64. Sharding: the c (query-row) axis of q/score/out is split
across 8 cores (256 rows each); k and v are replicated. The batch-axis softmax
needs, for every (c, c') pair, all 16 batch values - which all live on the same
core under c-sharding, so there are no collectives. Different c' columns are
independent, so we stream over c' in chunks of 128 and accumulate the second
matmul in PSUM.

Engine budget per c' chunk (the kernel is elementwise-bound, ACT is pacer):
  ACT   : 4x exp instrs (N=1024, PSUM->SBUF bf16)        ~4.4 us
  DVE   : tree L1 (A,Bt) + recip + cast + en=e*r         ~4.3 us
  GPSIMD: tree mid levels (C=A+Bt, D1, d)                ~3.9 us
  PE    : mm1 row-tiled 2x (K=64 pairs in T0/T8) and
          mm2 col-tiled 2x (v-stationary, M=64 pairs)    ~3.5 us cold
  DMA   : k chunk 0.5MB + v chunk 0.5MB                  ~2.9 us

mm1 row tiling: batches are laid out host-side as p = 64*h + l with
h = (b%4)//2, so each 4-batch PSUM group issues pairs (T0-half, T8-half)
that land in DIFFERENT psum banks (slot s=2h+r, bank=s//2) - concurrent
row-tile drains into one bank are fatal, this layout avoids them.

mm2 col tiling: v[c',l] is the stationary operand (64 weight cols), en
streams (N=256); batch pairs (2t, 2t+1) write partitions 0-63 / 64-127 of
pair-tile t. PSUM acc banks are pre-cleared by 4 dummy matmuls so every
real mm2 runs start=False (overwrite-where-clear handles chunk 0).
"""

import os

import numpy as np
import ml_dtypes

B, C, L = 16, 2048, 64
NCORES = 8
CB = C // NCORES  # 256 query rows per core
NJ = 16           # c' chunks of 128
P = 128

# debug bisect knobs (comma-separated): gp1 (tree mid-levels on GPSIMD —
# measured LOSS: GPSIMD shares the DVE SBUF port pair at ~4x the port-time
# per element), oldmm2 (en-stationary non-col-tiled mm2), flatmm1 (no
# row-tile pairing), nofastrecip (fp32 recip + separate bf16 cast)
_VARIANT = set(filter(None, os.environ.get("KERNEL_VARIANT", "").split(",")))

_NC_CACHE: dict = {}


def _build_nc():
    import concourse.mybir as mybir
    import concourse.tile as tile
    from concourse import bacc

    f32 = mybir.dt.float32
    bf16 = mybir.dt.bfloat16
    Exp = mybir.ActivationFunctionType.Exp
    ADD = mybir.AluOpType.add
    MUL = mybir.AluOpType.mult

    nc = bacc.Bacc(None, target_bir_lowering=False, debug=False)

    use_gp = "gp1" in _VARIANT
    oldmm2 = "oldmm2" in _VARIANT

    # qt[p, m, cq]: p = 64*h + l (h=(b%4)//2), m = 2*(b//4) + b%2
    qt = nc.declare_dram_parameter("qt", [P, 8, CB], bf16, isOutput=False)
    # kt[j, p, m, c']: same (p, m) mapping as qt
    kt = nc.declare_dram_parameter("kt", [NJ, P, 8, 128], bf16, isOutput=False)
    # vt[j, c', b, l]  (16, 128, 16, 64)
    vt = nc.declare_dram_parameter("vt", [NJ, P, B, L], bf16, isOutput=False)
    if oldmm2:
        # outd[t, cq_lo, bq, h, l]: b = 4t + bq, cq = 128h + cq_lo
        outd = nc.declare_dram_parameter("outd", [4, P, 4, 2, L], f32, isOutput=True)
    else:
        # outd[p, t, cq]: b = 2t + p//64, l = p%64
        outd = nc.declare_dram_parameter("outd", [P, 8, CB], f32, isOutput=True)

    with tile.TileContext(nc) as tc:
        with (
            tc.tile_pool(name="qp", bufs=1) as qp,
            tc.tile_pool(name="kp", bufs=4) as kp,
            tc.tile_pool(name="vp", bufs=4) as vp,
            tc.tile_pool(name="ep", bufs=4) as ep,
            tc.tile_pool(name="enp", bufs=3) as enp,
            tc.tile_pool(name="tp", bufs=3) as tp,
            tc.tile_pool(name="dp", bufs=3) as dp,
            tc.tile_pool(name="osp", bufs=4) as osp,
            tc.tile_pool(name="mm1p", bufs=2, space="PSUM") as mm1p,
            tc.tile_pool(name="accp", bufs=1, space="PSUM") as accp,
        ):
            qt_s = qp.tile([P, 8, CB], bf16)
            for g in range(4):
                nc.sync.dma_start(out=qt_s[:, 2 * g : 2 * g + 2], in_=qt[:, 2 * g : 2 * g + 2])

            if oldmm2:
                accs_old = [accp.tile([P, 4, 2, L], f32, name=f"acc{t}") for t in range(4)]
                accs = None
            else:
                accs = accp.tile([P, 8, CB], f32, name="acc")

            # HAM warmup: dense zero matmuls so the PE clock is at 8/8 by the
            # time real work lands; runs under the initial k/v DMAs.
            nwarm = int(os.environ.get("KERNEL_NWARM", "14"))
            wseed = qp.tile([P, 512], bf16, name="wseed")
            nc.vector.memset(wseed[:], 0)
            if nwarm:
                wps = mm1p.tile([P, 4, CB], f32, name="ps")
                for i in range(nwarm):
                    s = (0, 2, 1, 3)[i % 4]
                    h = s // 2
                    nc.tensor.matmul(
                        wps[:, s],
                        lhsT=wseed[64 * h : 64 * h + 64, :128],
                        rhs=wseed[64 * h : 64 * h + 64, :CB],
                        start=True,
                        stop=True,
                    )

            if not oldmm2:
                # pre-clear the 4 acc banks: one dummy start=True matmul per
                # bank clears its has_written bits, so all real mm2 matmuls
                # use start=False (overwrite-where-clear == accumulate-from-0)
                for u in range(4):
                    nc.tensor.matmul(
                        accs[:, 2 * u, 0:1],
                        lhsT=wseed[0:64, 0:128],
                        rhs=wseed[0:64, 0:1],
                        start=True,
                        stop=False,
                        skip_group_check=True,
                    )

            def emit_mm2_pair(j, en_j, v_j, t):
                # col-tiled pair: batch 2t -> out partitions 0-63 (tile (0,0)),
                # batch 2t+1 -> 64-127 (tile (0,64)); both into bank 4 + t//2.
                last = j == NJ - 1
                for p_ in range(2):
                    b = 2 * t + p_
                    nc.tensor.matmul(
                        accs[64 * p_ : 64 * p_ + 64, t, :],
                        lhsT=v_j[:, b],
                        rhs=en_j[:, b],
                        start=False,
                        stop=last and p_ == 1 and t % 2 == 1,
                        skip_group_check=True,
                    )

            def emit_mm2_b_old(j, en_j, v_j, b):
                acc = accs_old[b // 4]
                for h in range(2):
                    first_in_bank = j == 0 and b % 4 == 0 and h == 0
                    last_in_bank = j == NJ - 1 and b % 4 == 3 and h == 1
                    nc.tensor.matmul(
                        acc[:, b % 4, h],
                        lhsT=en_j[:, b, h * 128 : (h + 1) * 128],
                        rhs=v_j[:, b],
                        start=first_in_bank,
                        stop=last_in_bank,
                        skip_group_check=not (first_in_bank or last_in_bank),
                    )

            def emit_mm2(j, en_j, v_j):
                if oldmm2:
                    for b in range(B):
                        emit_mm2_b_old(j, en_j, v_j, b)
                else:
                    for t in range(8):
                        emit_mm2_pair(j, en_j, v_j, t)

            sorder = (0, 1, 2, 3) if "flatmm1" in _VARIANT else (0, 2, 1, 3)

            pending = None  # software pipeline: mm2 for chunk j-1 is emitted
            # during chunk j so the PE never waits on the softmax chain
            for j in range(NJ):
                k_j = kp.tile([P, 8, 128], bf16, name="k_j")
                nc.sync.dma_start(out=k_j[:], in_=kt[j])
                v_j = vp.tile([P, B, L], bf16, name="v_j")
                nc.sync.dma_start(out=v_j[:], in_=vt[j])

                e_j = ep.tile([P, B, CB], bf16, name="e_j")
                A_t = tp.tile([P, 4, CB], bf16, name="A_t")
                B_t = tp.tile([P, 4, CB], bf16, name="B_t")
                for g in range(4):
                    ps = mm1p.tile([P, 4, CB], f32, name="ps")
                    # slot s = 2h + r -> batch 4g + s; pairs (s=0,2) and
                    # (s=1,3) run concurrently on row tiles T0/T8 and write
                    # different psum banks (s//2).
                    for s in sorder:
                        h, r = s // 2, s % 2
                        nc.tensor.matmul(
                            ps[:, s],
                            lhsT=k_j[64 * h : 64 * h + 64, 2 * g + r],
                            rhs=qt_s[64 * h : 64 * h + 64, 2 * g + r],
                            start=True,
                            stop=True,
                        )
                    nc.scalar.activation(
                        e_j[:, 4 * g : 4 * g + 4], ps[:], Exp, scale=0.125
                    )
                    if g == 1:
                        nc.vector.tensor_tensor(
                            A_t[:], e_j[:, 0:4], e_j[:, 4:8], ADD
                        )
                if pending is not None:
                    emit_mm2(*pending)

                nc.vector.tensor_tensor(B_t[:], e_j[:, 8:12], e_j[:, 12:16], ADD)

                # mid tree levels on GPSIMD (idle engine) except at the tail,
                # where chain latency matters and DVE is free
                tteng = nc.gpsimd if (use_gp and j < NJ - 1) else nc.vector
                C_t = tp.tile([P, 4, CB], bf16, name="C_t")
                tteng.tensor_tensor(C_t[:], A_t[:], B_t[:], ADD)
                D_t = tp.tile([P, 2, CB], bf16, name="D_t")
                tteng.tensor_tensor(D_t[:], C_t[:, 0:2], C_t[:, 2:4], ADD)
                d_f = dp.tile([P, CB], f32, name="d_f")
                tteng.tensor_tensor(d_f[:], D_t[:, 0], D_t[:, 1], ADD)

                r_b = dp.tile([P, CB], bf16, name="r_b")
                if "nofastrecip" in _VARIANT:
                    r_f = dp.tile([P, CB], f32, name="r_f")
                    nc.vector.reciprocal_approx_fast(r_f[:], d_f[:])
                    nc.vector.tensor_copy(out=r_b[:], in_=r_f[:])
                else:
                    # reciprocal_approx_fast with bf16 output: the uop chain
                    # computes in fp32 (seed needs the fp32 bit layout of the
                    # INPUT only); the final write-path converts to bf16,
                    # saving a separate cast. Bypasses the bass-side fp32-out
                    # assert via _custom_dve.
                    from concourse.dve_ops import (
                        RECIP_APPROX_FAST_CONSTS,
                        RECIPROCAL_APPROX_FAST,
                    )

                    c_ = RECIP_APPROX_FAST_CONSTS
                    nc.vector._custom_dve(
                        RECIPROCAL_APPROX_FAST,
                        out=r_b[:],
                        in0=d_f[:],
                        s0=c_["s0"],
                        s1=c_["s1"],
                        imm2=c_["imm2"],
                    )

                en_j = enp.tile([P, B, CB], bf16, name="en_j")
                if j == NJ - 1:
                    # tail: normalize in two halves, each followed by its mm2
                    # block; each acc bank's psum->sbuf copy + store DMA is
                    # emitted as soon as that bank's accumulation completes
                    for gh in range(2):
                        nc.vector.tensor_tensor(
                            en_j[:, 8 * gh : 8 * gh + 8],
                            e_j[:, 8 * gh : 8 * gh + 8],
                            r_b[:, None, :].to_broadcast((P, 8, CB)),
                            MUL,
                        )
                        if oldmm2:
                            for b in range(8 * gh, 8 * gh + 8):
                                emit_mm2_b_old(j, en_j, v_j, b)
                                if b % 4 == 3:
                                    t = b // 4
                                    o_s = osp.tile([P, 4, 2, L], f32, name="o_s")
                                    if t % 2 == 0:
                                        nc.vector.tensor_copy(
                                            out=o_s[:], in_=accs_old[t][:]
                                        )
                                    else:
                                        nc.scalar.copy(o_s[:], accs_old[t][:])
                                    nc.sync.dma_start(out=outd[t], in_=o_s[:])
                        else:
                            for t in range(4 * gh, 4 * gh + 4):
                                emit_mm2_pair(j, en_j, v_j, t)
                                if t % 2 == 1:
                                    u = t // 2
                                    o_s = osp.tile([P, 2, CB], f32, name="o_s")
                                    if u % 2 == 0:
                                        nc.vector.tensor_copy(
                                            out=o_s[:],
                                            in_=accs[:, 2 * u : 2 * u + 2],
                                        )
                                    else:
                                        nc.scalar.copy(
                                            o_s[:], accs[:, 2 * u : 2 * u + 2]
                                        )
                                    nc.sync.dma_start(
                                        out=outd[:, 2 * u : 2 * u + 2], in_=o_s[:]
                                    )
                else:
                    nc.vector.tensor_tensor(
                        en_j[:],
                        e_j[:],
                        r_b[:, None, :].to_broadcast((P, B, CB)),
                        MUL,
                    )
                    pending = (j, en_j, v_j)

    nc.compile()
    return nc


def get_nc():
    if "nc" not in _NC_CACHE:
        _NC_CACHE["nc"] = _build_nc()
    return _NC_CACHE["nc"]


def make_in_maps(q, k, v):
    q = np.asarray(q, dtype=np.float32)
    k = np.asarray(k, dtype=np.float32)
    v = np.asarray(v, dtype=np.float32)

    qb = q.astype(ml_dtypes.bfloat16)
    kb = k.astype(ml_dtypes.bfloat16)
    # q (b, cq, l) -> qt[64h+l, 2g+r, cq] with b = 4g + 2h + r
    qt_all = np.ascontiguousarray(
        qb.reshape(4, 2, 2, C, L).transpose(1, 4, 0, 2, 3)
    ).reshape(P, 8, C)
    # k -> k_t[b, l, cfull] (row-major reshape) -> kt[j, 64h+l, 2g+r, c']
    ktt = np.ascontiguousarray(
        kb.reshape(4, 2, 2, L, NJ, 128).transpose(4, 1, 3, 0, 2, 5)
    ).reshape(NJ, P, 8, 128)
    # v -> bf16, (c', b, l) -> (j, c'128, b, l)
    vbt = np.ascontiguousarray(
        v.astype(ml_dtypes.bfloat16).transpose(1, 0, 2)
    ).reshape(NJ, P, B, L)

    in_maps = []
    for g in range(NCORES):
        in_maps.append(
            {
                "qt": np.ascontiguousarray(qt_all[:, :, g * CB : (g + 1) * CB]),
                "kt": ktt,
                "vt": vbt,
            }
        )
    return in_maps


def assemble_out(results):
    out = np.empty((B, C, L), dtype=np.float32)
    oldmm2 = "oldmm2" in _VARIANT
    for g in range(NCORES):
        od = np.asarray(results[g]["outd"])
        if oldmm2:
            oc = od.transpose(0, 2, 3, 1, 4).reshape(B, CB, L)
        else:
            # od[p, t, cq]: b = 2t + p//64, l = p%64
            oc = (
                od.reshape(2, L, 8, CB).transpose(2, 0, 3, 1).reshape(B, CB, L)
            )
        out[:, g * CB : (g + 1) * CB, :] = oc
    return out


def run(q, k, v, trace=False, trace_kwargs=None):
    """Run on 8 NeuronCores; returns (out, BassKernelResults)."""
    from concourse.bass_utils import run_bass_kernel_spmd

    nc = get_nc()
    in_maps = make_in_maps(q, k, v)
    kwargs = {}
    if trace:
        kwargs["trace"] = True
        if trace_kwargs:
            kwargs["trace_kwargs"] = trace_kwargs
    res = run_bass_kernel_spmd(nc, in_maps, core_ids=list(range(NCORES)), **kwargs)
    return assemble_out(res.results), res


def kernel(q, k, v):
    out, _ = run(q, k, v, trace=False)
    return out
